# revision 24
# baseline (speedup 1.0000x reference)
"""Trainium2 Bass kernel for nn_Attention4D (B=64, DIM=384, 14x14, 8 heads).

Sharding: pure data-parallel over batch. 8 items per NeuronCore, 8 cores,
weights replicated; inputs sharded/gathered on host.

v2 (engine-rebalanced) pipeline per item:
  - Q,K,V projections as f32r matmuls (pair-batched, N=392 moving).
  - V^T for attn@v obtained by PE *transposes* of the natural-layout V
    (16 x [128,98]->[98,128], 2048 PE cycles) instead of recomputing the
    V matmul in transposed form (6144 cycles).  V^T therefore includes
    vb, which together with the th2_b-in-sm2t trick removes the whole
    rowsum(v)/obias machinery of v1.
  - Scores PSUM tiles hold WINDOW PAIRS [112, 2, 196]: one bias-preload
    matmul and one exp() per pair (halves Act instruction count).
    Softmax denominators via DVE tensor_reduce (no Act accumulator reads).
  - t2r (th2^T kron I, row-scaled by 1/denom) built with per-partition
    tensor_scalar (4x DVE mode) - recip result used as f32 scalar AP.
  - th2+transpose matmul as in v1; sm2t eviction adds th2_b via
    scalar_tensor_tensor against a replicated th2_b tile, so attn@v
    computes th2@sm@v + th2_b*rowsum(v) in one DoubleRow fp8 matmul.
  - Depthwise 3x3: center + dy=0 + dx=0 taps on DVE (bf16 SBUF, 4x mode,
    vlb folded into the center tap), 4 diagonal taps as diag-stationary
    PE matmuls directly into the attn@v PSUM (i128-preloaded with vloc).
  - ReLU on eviction, final projection bf16, merged per-item output DMA.
"""

import sys

import numpy as np

sys.path.insert(0, "/opt/trn_rl_repo")

import ml_dtypes  # noqa: E402

import concourse.bacc as bacc  # noqa: E402
import concourse.mybir as mybir  # noqa: E402
import concourse.tile as tile  # noqa: E402
from concourse.bass_utils import run_bass_kernel_spmd  # noqa: E402

BF = mybir.dt.bfloat16
F32 = mybir.dt.float32
F32R = mybir.dt.float32r
F8 = mybir.dt.float8e4
DRM = mybir.MatmulPerfMode.DoubleRow
SVT = 32.0     # vt8 fp8 scale
SSM = 64.0     # sm2t fp8 scale  (attn@v PSUM scale = SVT*SSM = 2048)
PSC = SVT * SSM

B, C, RES = 64, 384, 14
NH, KD, DV = 8, 32, 128
N = RES * RES            # 196
SCALE = KD ** -0.5
NCORES = 8
BL = B // NCORES         # 8 items per core
NS = 14                  # query-window size
NW = N // NS             # 14 windows
P112 = NH * NS           # 112 partitions for attn tiles
IDENT = mybir.ActivationFunctionType.Identity
EXP = mybir.ActivationFunctionType.Exp
RELU = mybir.ActivationFunctionType.Relu
MULT = mybir.AluOpType.mult
ADD = mybir.AluOpType.add
AXX = mybir.AxisListType.X

_CACHE = {}


def _build_nc():
    nc = bacc.Bacc(None, target_bir_lowering=False)

    x8 = nc.declare_dram_parameter("x8", [BL, C, N], F32, isOutput=False)
    qkw_t = nc.declare_dram_parameter("qkw_t", [128, 3, 512], F32, isOutput=False)
    qkb_p = nc.declare_dram_parameter("qkb_p", [128, 4], F32, isOutput=False)
    vw_t = nc.declare_dram_parameter("vw_t", [128, 3, 1024], F32, isOutput=False)
    vb_p = nc.declare_dram_parameter("vb_p", [128, 8], F32, isOutput=False)
    vlw_p = nc.declare_dram_parameter("vlw_p", [128, 8, 9], F32, isOutput=False)
    vlb_p = nc.declare_dram_parameter("vlb_p", [128, 8], F32, isOutput=False)
    th1s_p = nc.declare_dram_parameter("th1s_p", [128, 2, 8], F32, isOutput=False)
    th2b_p = nc.declare_dram_parameter("th2b_p", [128, 8], F32, isOutput=False)
    t2k_d = nc.declare_dram_parameter("t2k", [P112, P112], BF, isOutput=False)
    bias1_d = nc.declare_dram_parameter("bias1_il", [P112, NW // 2, 2 * N], BF, isOutput=False)
    i112_d = nc.declare_dram_parameter("i112", [P112, P112], BF, isOutput=False)
    i128_d = nc.declare_dram_parameter("i128", [128, 128], BF, isOutput=False)
    projw_t = nc.declare_dram_parameter("projw_t", [128, 8, 384], BF, isOutput=False)
    dwdiag_d = nc.declare_dram_parameter("dwdiag", [128, 8, 6, 128], BF, isOutput=False)
    projb_p = nc.declare_dram_parameter("projb_p", [128, 3], F32, isOutput=False)

    y8 = nc.declare_dram_parameter("y8", [BL, C, N], F32, isOutput=True)

    with tile.TileContext(nc) as tc:
        with (
            tc.tile_pool(name="const", bufs=1) as const,
            tc.tile_pool(name="pairp", bufs=2) as pairp,
            tc.tile_pool(name="itemp", bufs=3) as itemp,
            tc.tile_pool(name="egrp", bufs=3) as epool,
            tc.tile_pool(name="t2rp", bufs=3) as t2rpool,
            tc.tile_pool(name="ps392", bufs=2, space="PSUM") as ps392,
            tc.tile_pool(name="psA", bufs=2, space="PSUM") as psA,
            tc.tile_pool(name="psUT", bufs=2, space="PSUM") as psUT,
            tc.tile_pool(name="psT", bufs=2, space="PSUM") as psT,
        ):
            # ---------------- early input prefetch (pair 0) ----------------
            x2_first = pairp.tile([128, 3, 2, N], F32R, tag="x2")
            for i2 in range(2):
                nc.sync.dma_start(
                    out=x2_first[:, :, i2],
                    in_=x8[i2].rearrange("(c p) n -> p c n", p=128).bitcast(F32R),
                )
            # ---------------- constants ----------------
            qkw_sb = const.tile([128, 3, 512], F32R)
            for c in range(3):
                nc.sync.dma_start(out=qkw_sb[:, c], in_=qkw_t[:, c].bitcast(F32R))
            vw_sb = const.tile([128, 3, 1024], F32R)
            for c in range(3):
                nc.sync.dma_start(out=vw_sb[:, c], in_=vw_t[:, c].bitcast(F32R))
            qkb_sb = const.tile([128, 4], F32)
            nc.sync.dma_start(out=qkb_sb[:], in_=qkb_p[:])
            vb_sb = const.tile([128, 8], F32)
            nc.sync.dma_start(out=vb_sb[:], in_=vb_p[:])
            vlw_sb = const.tile([128, 8, 9], F32)
            nc.sync.dma_start(out=vlw_sb[:], in_=vlw_p[:])
            vlb_sb = const.tile([128, 8], F32)
            nc.sync.dma_start(out=vlb_sb[:], in_=vlb_p[:])
            th1s_sb = const.tile([128, 2, 8], F32)
            nc.sync.dma_start(out=th1s_sb[:], in_=th1s_p[:])
            th2b_sb = const.tile([128, 8], F32)
            nc.sync.dma_start(out=th2b_sb[:], in_=th2b_p[:])
            projb_sb = const.tile([128, 3], F32)
            nc.sync.dma_start(out=projb_sb[:], in_=projb_p[:])

            t2k_sb = const.tile([P112, P112], BF)
            nc.sync.dma_start(out=t2k_sb[:], in_=t2k_d[:])
            i112_sb = const.tile([P112, P112], BF)
            nc.sync.dma_start(out=i112_sb[:], in_=i112_d[:])
            i128_sb = const.tile([128, 128], BF)
            nc.sync.dma_start(out=i128_sb[:], in_=i128_d[:])
            projw_sb = const.tile([128, 8, 384], BF)
            nc.sync.dma_start(out=projw_sb[:], in_=projw_t[:])
            dwdiag_sb = const.tile([128, 8, 6, 128], BF)
            nc.sync.dma_start(out=dwdiag_sb[:], in_=dwdiag_d[:])
            bias1_sb = const.tile([P112, NW // 2, 2 * N], BF)
            nc.sync.dma_start(out=bias1_sb[:], in_=bias1_d[:])

            # ---------------- per item-pair ----------------
            for pr in range(BL // 2):
                if pr == 0:
                    x2 = x2_first
                else:
                    x2 = pairp.tile([128, 3, 2, N], F32R, tag="x2")
                    for i2 in range(2):
                        nc.sync.dma_start(
                            out=x2[:, :, i2],
                            in_=x8[2 * pr + i2]
                            .rearrange("(c p) n -> p c n", p=128)
                            .bitcast(F32R),
                        )

                # --- Q,K projections: chunks mt 0,1 = q; 2,3 = k ---
                qk = pairp.tile([128, 4, 2, N], BF, tag="qk")
                for mt in range(4):
                    pp = ps392.tile([128, 392], F32, tag="mm392")
                    for c in range(3):
                        nc.tensor.matmul(
                            pp[:],
                            qkw_sb[:, c, mt * 128 : (mt + 1) * 128],
                            x2[:, c].rearrange("p i n -> p (i n)"),
                            start=(c == 0),
                            stop=(c == 2),
                        )
                    if mt < 2:
                        nc.vector.tensor_scalar(
                            qk[:, mt].rearrange("p i n -> p (i n)"),
                            pp[:],
                            qkb_sb[:, mt : mt + 1],
                            None,
                            ADD,
                        )
                    else:
                        nc.scalar.activation(
                            qk[:, mt].rearrange("p i n -> p (i n)"),
                            pp[:],
                            IDENT,
                            bias=qkb_sb[:, mt : mt + 1],
                            scale=1.0,
                        )

                # --- V projection, natural layout; +vb at evict (Act) ---
                avb = pairp.tile([128, 8, 2, N], BF, tag="avb")
                vsum = pairp.tile([128, 8, 2], F32, tag="vsum")
                obias = pairp.tile([128, 8, 2], F32, tag="obias")
                for ch in range(8):
                    pp = ps392.tile([128, 392], F32, tag="mm392")
                    for c in range(3):
                        nc.tensor.matmul(
                            pp[:],
                            vw_sb[:, c, ch * 128 : (ch + 1) * 128],
                            x2[:, c].rearrange("p i n -> p (i n)"),
                            start=(c == 0),
                            stop=(c == 2),
                        )
                    nc.scalar.activation(
                        avb[:, ch].rearrange("p i n -> p (i n)"),
                        pp[:],
                        IDENT,
                        bias=vb_sb[:, ch : ch + 1],
                        scale=1.0,
                    )
                    # obias = rowsum(v)*th2_b[h]  (v incl vb rides vt8/sm2t)
                    nc.vector.tensor_reduce(
                        vsum[:, ch], avb[:, ch], AXX, ADD
                    )
                    nc.vector.tensor_scalar(
                        obias[:, ch],
                        vsum[:, ch],
                        th2b_sb[:, ch : ch + 1],
                        None,
                        MULT,
                    )

                # --- depthwise 3x3: 3 of 9 taps on DVE into vloc (x2048) ---
                # center tap (4x tensor_scalar) initializes with vlb folded
                # in; dy=0,dx=+-1 accumulate (scalar_tensor_tensor).  The 6
                # dy=+-1 taps ride PE as diag-stationary matmuls later.
                vloc = pairp.tile([128, 8, 2, N], BF, tag="vloc")
                for ch in range(8):
                    nc.vector.tensor_scalar(
                        vloc[:, ch].rearrange("p i n -> p (i n)"),
                        avb[:, ch].rearrange("p i n -> p (i n)"),
                        vlw_sb[:, ch, 4:5],
                        vlb_sb[:, ch : ch + 1],
                        MULT,
                        ADD,
                    )
                    sr = avb[:, ch].rearrange("p i (y x) -> p (i y) x", x=RES)
                    dr = vloc[:, ch].rearrange("p i (y x) -> p (i y) x", x=RES)
                    for dx in (-1, 1):  # dy == 0: (i y) merged, both items
                        t = 3 + (dx + 1)
                        x0, x1 = max(0, -dx), min(RES, RES - dx)
                        d = dr[:, :, x0:x1]
                        s = sr[:, :, x0 + dx : x1 + dx]
                        nc.vector.scalar_tensor_tensor(
                            d, s, vlw_sb[:, ch, t : t + 1], d, MULT, ADD
                        )

                orelu = pairp.tile([128, 8, 2, N], BF, tag="orelu")
                sm2ts = []
                vts = []

                for i2 in range(2):
                    # --- V^T via PE transposes of avb (includes vb) ---
                    vt8 = itemp.tile([98, 2, 1024], F8, tag="vta")
                    for half in range(2):
                        for hg in range(2):
                            pt = psT.tile([98, 4, 128], BF, tag="vtps")
                            for hj in range(4):
                                h = hg * 4 + hj
                                nc.tensor.transpose(
                                    pt[:, hj],
                                    avb[:, h, i2, half * 98 : half * 98 + 98],
                                    i128_sb[:],
                                )
                            nc.scalar.activation(
                                vt8[:, half, hg * 512 : (hg + 1) * 512],
                                pt[:].rearrange("p a b -> p (a b)"),
                                IDENT,
                                bias=0.0,
                                scale=SVT,
                            )

                    # --- th1-scaled Q copies, window-major: [c2, w, (h,ns)] ---
                    qp = itemp.tile([128, 2, NW, NH, NS], BF, tag="qp")
                    for c2 in range(2):
                        for g in range(NH):
                            nc.gpsimd.tensor_scalar(
                                qp[:, c2, :, g, :],
                                qk[:, c2, i2].rearrange(
                                    "p (w ns) -> p w ns", ns=NS
                                ),
                                th1s_sb[:, c2, g : g + 1],
                                None,
                                MULT,
                            )

                    # --- per window-pair: scores+exp, recip, t2r, th2 ---
                    dsum = itemp.tile([P112, NW], F32, tag="dsum")
                    rbuf = itemp.tile([P112, NW], F32, tag="rbuf")
                    sm2t8 = itemp.tile([98, 2, NW, NH, NS], F8, tag="sm2a")
                    for wp in range(NW // 2):
                        w0 = 2 * wp
                        sp = psA.tile([P112, 2, N], F32, tag="attnps")
                        nc.tensor.matmul(
                            sp[:].rearrange("p a n -> p (a n)"),
                            i112_sb[:],
                            bias1_sb[:, wp],
                            start=True,
                            stop=False,
                        )
                        for wi in range(2):
                            for c2 in range(2):
                                nc.tensor.matmul(
                                    sp[:, wi],
                                    qp[:, c2, w0 + wi],
                                    qk[:, 2 + c2, i2],
                                    start=False,
                                    stop=(c2 == 1),
                                )
                        egr = epool.tile([P112, 2, N], BF, tag="egrp")
                        nc.scalar.activation(
                            egr[:].rearrange("p a n -> p (a n)"),
                            sp[:].rearrange("p a n -> p (a n)"),
                            EXP,
                        )
                        nc.vector.tensor_reduce(
                            dsum[:, w0 : w0 + 2], egr[:], AXX, ADD
                        )
                        nc.vector.reciprocal(
                            rbuf[:, w0 : w0 + 2], dsum[:, w0 : w0 + 2]
                        )
                        t2r = t2rpool.tile([P112, 2, P112], BF, tag="t2r")
                        ut = psUT.tile([98, 2, 2, P112], F32, tag="utps")
                        for wi in range(2):
                            nc.gpsimd.tensor_scalar(
                                t2r[:, wi],
                                t2k_sb[:],
                                rbuf[:, w0 + wi : w0 + wi + 1],
                                None,
                                MULT,
                            )
                            nc.tensor.matmul(
                                ut[:, 0, wi],
                                egr[:, wi, 0:98],
                                t2r[:, wi],
                                start=True,
                                stop=True,
                            )
                            nc.tensor.matmul(
                                ut[:, 1, wi],
                                egr[:, wi, 98:N],
                                t2r[:, wi],
                                start=True,
                                stop=True,
                            )
                        # evict * SSM -> fp8
                        for half in range(2):
                            nc.vector.tensor_scalar(
                                sm2t8[:, half, w0 : w0 + 2].rearrange(
                                    "m w h ns -> m w (h ns)"
                                ),
                                ut[:, half],
                                SSM,
                                None,
                                MULT,
                            )

                    sm2ts.append(sm2t8)
                    vts.append(vt8)

                # --- attn@v + vloc preload + PE dy=+-1 taps; ReLU evict ---
                for h in range(NH):
                    op = ps392.tile([128, 2, N], F32, tag="mm392")
                    nc.tensor.matmul(
                        op[:].rearrange("p i n -> p (i n)"),
                        i128_sb[:],
                        vloc[:, h].rearrange("p i n -> p (i n)"),
                        start=True,
                        stop=False,
                    )
                    for slot, (dy, dx) in enumerate(
                        [(-1, -1), (-1, 0), (-1, 1), (1, -1), (1, 0), (1, 1)]
                    ):
                        y0, y1 = max(0, -dy), min(RES, RES - dy)
                        x0, x1 = max(0, -dx), min(RES, RES - dx)
                        if dx == 0:
                            d = op[:].rearrange("p i (y x) -> p i (y x)", x=RES)[
                                :, :, y0 * RES : y1 * RES
                            ]
                            s = avb[:, h][:, :, (y0 + dy) * RES : (y1 + dy) * RES]
                            nc.tensor.matmul(
                                d, dwdiag_sb[:, h, slot], s, start=False, stop=False
                            )
                        else:
                            for i2 in range(2):
                                d = op[:, i2].rearrange("p (y x) -> p y x", x=RES)[
                                    :, y0:y1, x0:x1
                                ]
                                s = avb[:, h, i2].rearrange(
                                    "p (y x) -> p y x", x=RES
                                )[:, y0 + dy : y1 + dy, x0 + dx : x1 + dx]
                                nc.tensor.matmul(
                                    d,
                                    dwdiag_sb[:, h, slot],
                                    s,
                                    start=False,
                                    stop=False,
                                )
                    for i2 in range(2):
                        nc.tensor.matmul(
                            op[:, i2].rearrange("p (w ns) -> p w ns", ns=NS),
                            vts[i2][:, :, h * 128 : (h + 1) * 128],
                            sm2ts[i2][:, :, :, h],
                            start=False,
                            stop=(i2 == 1),
                            perf_mode=DRM,
                        )
                    for i2 in range(2):
                        nc.scalar.activation(
                            orelu[:, h, i2],
                            op[:, i2],
                            RELU,
                            bias=obias[:, h, i2 : i2 + 1],
                            scale=1.0 / PSC,
                        )

                # --- final projection (pair-wide, bf16) ---
                out_sb = pairp.tile([128, 3, 2, N], F32, tag="out")
                for mt in range(3):
                    pp = ps392.tile([128, 392], F32, tag="mm392")
                    for ch in range(8):
                        nc.tensor.matmul(
                            pp[:],
                            projw_sb[:, ch, mt * 128 : (mt + 1) * 128],
                            orelu[:, ch].rearrange("p i n -> p (i n)"),
                            start=(ch == 0),
                            stop=(ch == 7),
                        )
                    nc.scalar.activation(
                        out_sb[:, mt].rearrange("p i n -> p (i n)"),
                        pp[:],
                        IDENT,
                        bias=projb_sb[:, mt : mt + 1],
                        scale=1.0,
                    )
                for i2 in range(2):
                    nc.sync.dma_start(
                        out=y8[2 * pr + i2].rearrange("(mt p) n -> p mt n", p=128),
                        in_=out_sb[:, :, i2],
                    )

    nc.compile()
    return nc


def _host_prep(qw, qb, kw, kb, vw, vb, vlw, vlb, th1_w, th1_b, th2_w, th2_b,
               projw, projb, bias_seg, bias_idxs):
    f = np.float32
    qw, qb, kw, kb = (np.asarray(a, f) for a in (qw, qb, kw, kb))
    vw, vb, vlw, vlb = (np.asarray(a, f) for a in (vw, vb, vlw, vlb))
    th1_w, th1_b, th2_w, th2_b = (
        np.asarray(a, f) for a in (th1_w, th1_b, th2_w, th2_b)
    )
    projw, projb = np.asarray(projw, f), np.asarray(projb, f)
    bias_seg = np.asarray(bias_seg, f)
    bias_idxs = np.asarray(bias_idxs)

    qkw = np.concatenate([qw * SCALE, kw], axis=0)                     # [512,384]
    qkw_t = np.ascontiguousarray(qkw.T.reshape(3, 128, 512).transpose(1, 0, 2))
    qkb = np.concatenate([qb * SCALE, kb])
    qkb_p = np.ascontiguousarray(qkb.reshape(4, 128).T)

    vw_t = np.ascontiguousarray(vw.T.reshape(3, 128, 1024).transpose(1, 0, 2))
    vb_p = np.ascontiguousarray(vb.reshape(8, 128).T)

    vlw9 = vlw.reshape(1024, 9)
    vlw_p = np.ascontiguousarray(
        vlw9.reshape(8, 128, 9).transpose(1, 0, 2)
    ) * PSC
    vlb_p = np.ascontiguousarray(vlb.reshape(8, 128).T) * PSC

    th1s = np.repeat(th1_w.T, KD, axis=0)                              # [256,8]
    th1s_p = np.ascontiguousarray(th1s.reshape(2, 128, 8).transpose(1, 0, 2))

    t2k = np.kron(th2_w.T, np.eye(NS, dtype=f))                        # [112,112]
    th2b_p = np.ascontiguousarray(np.repeat(th2_b[:, None], 128, axis=1).T)

    bias_full = bias_seg[:, bias_idxs]                                 # [8,196,196]
    bias1 = np.einsum("hg,gnm->hnm", th1_w, bias_full)
    bias1 += th1_b[:, None, None]
    bias1_il = np.ascontiguousarray(
        bias1.reshape(NH, NW, NS, N).transpose(0, 2, 1, 3).reshape(P112, NW, N)
    ).reshape(P112, NW // 2, 2 * N)

    projw_t = np.ascontiguousarray(
        projw.T.reshape(8, 128, 384).transpose(1, 0, 2)
    )

    # diag-stationary weights for the 6 dy=+-1 dwconv taps on TensorE
    taps6 = [0, 1, 2, 6, 7, 8]  # (dy,dx): (-1,-1),(-1,0),(-1,1),(1,-1),(1,0),(1,1)
    dwdiag = np.zeros((128, 8, 6, 128), f)
    eye = np.eye(128, dtype=f)
    for ch in range(8):
        for si, t in enumerate(taps6):
            dwdiag[:, ch, si, :] = (
                eye * vlw9[ch * 128 : (ch + 1) * 128, t][:, None] * PSC
            )
    projb_p = np.ascontiguousarray(projb.reshape(3, 128).T)

    nbf = ml_dtypes.bfloat16
    return dict(
        qkw_t=qkw_t, qkb_p=qkb_p, vw_t=vw_t, vb_p=vb_p, vlw_p=vlw_p,
        vlb_p=vlb_p, th1s_p=th1s_p, th2b_p=th2b_p,
        t2k=t2k.astype(nbf), bias1_il=bias1_il.astype(nbf),
        dwdiag=dwdiag.astype(nbf), i112=np.eye(P112, dtype=nbf),
        i128=np.eye(128, dtype=nbf), projw_t=projw_t.astype(nbf),
        projb_p=projb_p,
    )


def kernel(**inputs):
    x = np.asarray(inputs["x"], np.float32)
    consts = _host_prep(
        inputs["qw"], inputs["qb"], inputs["kw"], inputs["kb"],
        inputs["vw"], inputs["vb"], inputs["vlw"], inputs["vlb"],
        inputs["th1_w"], inputs["th1_b"], inputs["th2_w"], inputs["th2_b"],
        inputs["projw"], inputs["projb"], inputs["bias_seg"], inputs["bias_idxs"],
    )
    if "nc" not in _CACHE:
        _CACHE["nc"] = _build_nc()
    nc = _CACHE["nc"]

    xs = np.ascontiguousarray(x.reshape(NCORES, BL, C, N))
    in_maps = [dict(consts, x8=xs[i]) for i in range(NCORES)]
    res = run_bass_kernel_spmd(
        nc, in_maps, list(range(NCORES)), **_CACHE.get("run_kwargs", {})
    )
    _CACHE["last_results"] = res
    out = np.stack([np.asarray(res.results[i]["y8"]) for i in range(NCORES)])
    return out.reshape(B, C, RES, RES).astype(np.float32)
